# revision 19
# baseline (speedup 1.0000x reference)
"""Causal self-attention (GPT-2 block) for Trainium2, 8 NeuronCores.

Sharding: core = 2*batch + head_group. Each of the 8 cores handles one of
B=4 batches and one group of 8 of the 16 heads (Megatron column-split of
the QKV weights, row-split of the proj weights). The two head-group
partial proj outputs per batch are summed on the host; the V-bias and
proj-bias terms are folded into a single host-side additive correction
(softmax rows sum to 1, so attn @ (1 x bv) == bv broadcast).

The host ships x pre-transposed ([D, S], bf16) so the kernel needs no
PE transposes: xT chunks DMA straight into SBUF in the [d-partition,
s-free] layout every QKV matmul wants.

On-core layout (bf16 matmul operands; PE at 1 cycle/row):
  xT    [128, 8, S]   d-major x, DMA-loaded per 256-wide s-chunk
  QT/KT [128, 4, S]   feature-major: partition p, slice j <-> feature j*128+p
                      head h lives at partitions (h%2)*64.. , slice h//2
  V     [128, 16, 8, 65]  natural [s, feat] per head + ones column (row sums)
  attnT [128, 4, S]   attention output, feature-major (proj stationary)

Attention per (head-pair, chunk): scoresT blocks [128 k, <=W q] via
KT-block.T @ QT (contract 64; the pair occupies array halves 0/64 via
tile_position and runs CONCURRENTLY - verified dstart ~3ns on HW)
written into one 2-bank psum tile [128, 2, W]; additive -1e30 causal
mask on the diagonal corner; ONE fused exp on ScalarE for both heads
(1/8 scale folded in) amortizing the ~352-cycle ACTIVATE overhead; PV +
row-sums via the V ones-column; then a parallel-lane reciprocal
(DMA-scattered to [128, W/128]) and a DRAM-bounced partition broadcast.

Scheduling: one interleaved stream - ready attention chunks and proj
are dripped between the QKV matmul groups so the PE HAM activity
monitor stays above threshold (2.4 GHz). Tail thunks (attention of the
last chunk, proj, drain/norm) fire full-array junk matmuls to keep HAM
from dropping the PE clock to 1.2 GHz (measured 89us cold-tail
without; half-array attention matmuls alone do not register enough
activity).
"""

import os

import numpy as np

import concourse.bass as bass
import concourse.tile as tile
from concourse import bacc, mybir
from concourse.bass_utils import run_bass_kernel_spmd
from concourse.masks import make_identity, make_lower_triangular

# Problem shape (fixed by the harness contract).
B, S, D, H, HD = 4, 2048, 1024, 16, 64
NCORES = 8
HG = 8                # heads per core
FG = HG * HD          # 512 features per head group
P = 128
DB = D // P           # 8 contraction blocks
FBN = FG // P         # 4 feature blocks
SC = 512              # max attention sequence chunk
# Uneven attention chunks: (start, width). Narrow tail chunks shrink the
# non-overlappable ScalarE exp tail.
CHUNKS = [(0, 512), (512, 512), (1024, 512), (1536, 512)]
NCH = len(CHUNKS)
NKB = S // P          # 16 key blocks
F32 = mybir.dt.float32
F32R = mybir.dt.float32r
BF16 = mybir.dt.bfloat16
DT_MM = F32R if os.environ.get("KERNEL_DT") == "f32r" else BF16
EXP = mybir.ActivationFunctionType.Exp
SCALE = 1.0 / float(HD) ** 0.5
MASKVAL = -1e30


class _Ctx:
    """Tiles/pools shared by the emission thunks."""


def _attention_pair_thunks(nc, cx, hA, hB, ci, warm=False):
    """Thunks emitting one q-chunk of attention for a pair of heads.
    QT/attnT are per-chunk tiles (cx.QTc[ci], cx.attnTc[ci]). With
    warm=True each block also fires a junk full-array matmul (tail HAM
    warming)."""
    a, W = CHUNKS[ci]
    nfull = a // P
    ndiag = W // P
    # Full-width band blocks first (block 0 carries start=True), then the
    # diagonal k-blocks with narrowing width.
    blocks = [(kb, None) for kb in range(nfull)] + \
             [(nfull + jj, jj) for jj in range(ndiag)]
    nblk = len(blocks)
    st = {}

    def setup():
        st["heads"] = []
        for h in (hA, hB):
            out_ps = cx.psout.tile([65, SC], F32, tag="outps")
            st["heads"].append((h, (h % 2) * 64, h // 2, out_ps))

    def make_blk(i, kb, jj):
        def run():
            heads = st["heads"]
            off = 0 if jj is None else jj * P
            w = W - off
            stp = cx.psst.tile([P, 2, SC], F32, tag="stps")
            for hi, (h, pb, j, out_ps) in enumerate(heads):
                nc.tensor.matmul(
                    stp[:, hi, :w],
                    cx.KT[pb:pb + 64, j, kb * P:(kb + 1) * P],
                    cx.QTc[ci][pb:pb + 64, j, off:W],
                    start=True, stop=True, tile_position=(pb, 0))
            if warm:
                # Wider junk stream (256 cols of resident KT data): the
                # 128-col version left full-array duty ~5% per block,
                # marginal against the HAM activity threshold (tail
                # still oscillated cold ~18us).
                jp = cx.ps1.tile([P, 2 * P], F32, tag="qkps")
                nc.tensor.matmul(jp, cx.identb, cx.KT[:, 0, 0:2 * P],
                                 start=True, stop=True)
            if jj is not None:
                for hi in range(2):
                    nc.vector.tensor_add(
                        stp[:, hi, :P], stp[:, hi, :P], cx.addmask)
            sx = cx.sxp.tile([P, 2, SC], DT_MM, tag="sx")
            nc.scalar.activation(sx[:, :, :w], stp[:, :, :w], EXP,
                                 scale=SCALE)
            for hi, (h, pb, j, out_ps) in enumerate(heads):
                nc.tensor.matmul(
                    out_ps[:, off:W], cx.V[:, kb, h, :], sx[:, hi, :w],
                    start=(i == 0), stop=(i == nblk - 1))
        return run

    def drain():
        # Bulk-copy PV psum to SBUF: frees the psout bank immediately so
        # the next pair's PV is not gated on this pair's norm DMA round
        # trips (measured: direct-psum norm reads stall PE + HAM-cold).
        st["raws"] = []
        for h, pb, j, out_ps in st["heads"]:
            raw = cx.nrmraw.tile([65, SC], F32, tag="raw")
            nc.vector.tensor_copy(raw[:, :W], out_ps[:, :W])
            st["raws"].append(raw)
        if warm:
            jp = cx.ps1.tile([P, P], F32, tag="qkps")
            nc.tensor.matmul(jp, cx.identb, cx.identb,
                             start=True, stop=True)

    def norm():
        for (h, pb, j, out_ps), raw in zip(st["heads"], st["raws"]):
            # Single-partition reciprocal blocks the DVE FIFO for ~us;
            # DMA-scatter the sums across 128 partitions first.
            rsh = cx.nrmbc.tile([P, SC // P], F32, tag="rsh")
            nc.sync.dma_start(rsh[:, :W // P], raw[64:65, :W])
            nc.vector.reciprocal(rsh[:, :W // P], rsh[:, :W // P])
            rdram = cx.drp.tile([1, SC], F32, tag="rdram")
            nc.sync.dma_start(rdram[:, :W], rsh[:, :W // P])
            rb = cx.nrmbc.tile([64, SC], F32, tag="rb")
            nc.sync.dma_start(rb[:, :W], rdram[:, :W].to_broadcast([64, W]))
            stg = cx.nrmbc.tile([64, SC], DT_MM, tag="stg")
            nc.vector.tensor_mul(stg[:, :W], raw[0:64, :W], rb[:, :W])
            if warm:
                jp = cx.ps1.tile([P, P], F32, tag="qkps")
                nc.tensor.matmul(jp, cx.identb, cx.identb,
                                 start=True, stop=True)
            nc.sync.dma_start(cx.attnTc[ci][pb:pb + 64, j, :W], stg[:, :W])

    thunks = [setup]
    thunks += [make_blk(i, kb, jj) for i, (kb, jj) in enumerate(blocks)]
    thunks += [drain, norm]
    return thunks


def _attention_chunk_thunks(nc, cx, ci, warm=False):
    out = []
    for hp in range(HG // 2):
        out += _attention_pair_thunks(nc, cx, 2 * hp, 2 * hp + 1, ci,
                                      warm=warm)
    return out


def _proj_chunk_thunks(nc, cx, ci, out_d, warm=False):
    """Proj for the s-blocks of chunk ci; two thunks per s-block."""
    a, W = CHUNKS[ci]
    thunks = []
    for sb in range(W // P):
        sblk = a // P + sb

        def make_half(hf, sblk=sblk, sb=sb):
            def run():
                og = cx.ogp.tile([P, D // 2], F32, tag="og")
                ps = cx.ps1.tile([P, D // 2], F32, tag="qkps")
                n0 = hf * (D // 2)
                for j in range(FBN):
                    nc.tensor.matmul(
                        ps,
                        cx.attnTc[ci][:, j, sb * P:(sb + 1) * P],
                        cx.wp_sb[:, j, n0:n0 + D // 2],
                        start=(j == 0), stop=(j == FBN - 1))
                if warm:
                    jp = cx.ps1.tile([P, P], F32, tag="qkps")
                    nc.tensor.matmul(jp, cx.identb, cx.identb,
                                     start=True, stop=True)
                nc.any.tensor_copy(og, ps)
                nc.sync.dma_start(
                    out_d.ap()[sblk * P:(sblk + 1) * P, n0:n0 + D // 2], og)
            return run

        thunks.append(make_half(0))
        thunks.append(make_half(1))
    return thunks


def _body(tc, xT_d, wq_d, wk_d, wv_d, wp_d, bq_d, out_d):
    nc = tc.nc
    cx = _Ctx()
    XC = 256                  # QKV s-chunk width
    NXC = S // XC             # 8
    with (
        tc.tile_pool(name="persist", bufs=1) as persist,
        tc.tile_pool(name="ph1", bufs=1) as ph1,
        tc.tile_pool(name="xtp", bufs=2) as xtp,
        tc.tile_pool(name="qtc", bufs=3) as qtc,
        tc.tile_pool(name="atc", bufs=3) as atc,
        tc.tile_pool(name="sxp", bufs=3) as sxp,
        tc.tile_pool(name="nrmraw", bufs=3) as nrmraw,
        tc.tile_pool(name="nrmbc", bufs=2) as nrmbc,
        tc.tile_pool(name="ogp", bufs=2) as ogp,
        # PSUM banks: qkps 2 + stps (2-bank pair tiles) 4 + outps 2 = 8
        tc.tile_pool(name="ps1", bufs=2, space="PSUM") as ps1,
        tc.tile_pool(name="psst", bufs=2, space="PSUM") as psst,
        tc.tile_pool(name="psout", bufs=2, space="PSUM") as psout,
        tc.tile_pool(name="drp", bufs=8, space="DRAM") as drp,
    ):
        cx.sxp, cx.nrmraw, cx.nrmbc, cx.ogp = sxp, nrmraw, nrmbc, ogp
        cx.psst, cx.psout, cx.drp, cx.ps1 = psst, psout, drp, ps1

        ident = persist.tile([P, P], F32)
        make_identity(nc, ident)
        cx.identb = persist.tile([P, P], DT_MM)
        nc.vector.tensor_copy(cx.identb, ident)
        cx.addmask = persist.tile([P, P], F32)
        make_lower_triangular(nc, cx.addmask, val=MASKVAL, diag=False)
        bq_sb = persist.tile([P, FBN], F32)
        nc.sync.dma_start(bq_sb, bq_d.ap().rearrange("(j p) -> p j", p=P))

        cx.KT = persist.tile([P, FBN, S], DT_MM)
        cx.V = persist.tile([P, NKB, HG, HD + 1], DT_MM)
        ones_col = persist.tile([P, 1], F32)
        nc.vector.memset(ones_col, 1.0)
        nc.vector.tensor_copy(cx.V[:, :, :, HD],
                              ones_col.to_broadcast([P, NKB, HG]))
        cx.wp_sb = persist.tile([P, FBN, D], DT_MM)
        cx.QTc = [qtc.tile([P, FBN, SC], DT_MM, tag="qtc",
                           name=f"qtc{ci}") for ci in range(NCH)]
        cx.attnTc = [atc.tile([P, FBN, SC], DT_MM, tag="atc",
                              name=f"atc{ci}") for ci in range(NCH)]

        wq_sb = ph1.tile([P, DB, FG], DT_MM)
        wk_sb = ph1.tile([P, DB, FG], DT_MM)
        wv_sb = ph1.tile([P, DB, FG], DT_MM)

        xts = [xtp.tile([P, DB, XC], DT_MM, tag="xt", name=f"xt{xc}")
               for xc in range(NXC)]
        xTr = xT_d.ap().rearrange("(db p) s -> p db s", p=P)

        def load_chunk(xc):
            nc.sync.dma_start(xts[xc], xTr[:, :, xc * XC:(xc + 1) * XC])

        bg = []          # attention/proj thunks dripped between QKV groups

        def drip(nbg):
            for _ in range(nbg):
                if bg:
                    bg.pop(0)()

        load_chunk(0)
        # Paced pre-warm: junk full-array matmuls keep the PE HAM busy
        # through the initial weight/x load window.
        for _ in range(10):
            jp = ps1.tile([P, P], F32, tag="qkps")
            nc.tensor.matmul(jp, ident, ident, start=True, stop=True)
        # One DMA per weight tensor: the startup was dispatch-bound (27
        # small DMAs at ~600ns dispatch each serialize on the sync
        # queue); a single 3D-AP load per tensor cuts dispatch count.
        for w_sb, w_d in ((wq_sb, wq_d), (wk_sb, wk_d), (wv_sb, wv_d)):
            nc.sync.dma_start(
                w_sb, w_d.ap().rearrange("(db p) f -> p db f", p=P))
        nc.sync.dma_start(
            cx.wp_sb, wp_d.ap().rearrange("(j p) n -> p j n", p=P))

        # Which (s-range-complete) attention/proj chunks become ready at
        # the START of each xc iteration (QKV through xc-1 done).
        att_at = {2: [0], 4: [1], 6: [2]}
        proj_at = {4: [0], 6: [1]}

        # QTc chunk/slice mapping: chunk ci covers s in [a, a+W); xc
        # covers s in [xc*XC, (xc+1)*XC).
        def qt_dst(fb, xc):
            s0 = xc * XC
            for ci, (a, W) in enumerate(CHUNKS):
                if a <= s0 < a + W:
                    return cx.QTc[ci][:, fb, s0 - a:s0 - a + XC]
            raise AssertionError

        for xc in range(NXC):
            xt = xts[xc]
            if xc + 1 < NXC:
                load_chunk(xc + 1)
            for ci in att_at.get(xc, []):
                bg += _attention_chunk_thunks(nc, cx, ci)
            for ci in proj_at.get(xc, []):
                bg += _proj_chunk_thunks(nc, cx, ci, out_d)
            per = (len(bg) + 9) // 10

            # Q and K -> transposed feature-major layout; Q gets its
            # bias, K's bias is dropped (it adds a per-q constant to each
            # score row, which softmax is invariant to).
            for w_sb, is_q in ((wq_sb, True), (wk_sb, False)):
                for fb in range(FBN):
                    ps = ps1.tile([P, XC], F32, tag="qkps")
                    for db in range(DB):
                        nc.tensor.matmul(
                            ps,
                            w_sb[:, db, fb * P:(fb + 1) * P],
                            xt[:, db, :],
                            start=(db == 0), stop=(db == DB - 1))
                    if is_q:
                        nc.vector.tensor_scalar_add(
                            qt_dst(fb, xc), ps, bq_sb[:, fb:fb + 1])
                    else:
                        nc.vector.tensor_copy(
                            cx.KT[:, fb, xc * XC:(xc + 1) * XC], ps)
                    drip(per)
            # V -> natural [s, feat] layout (no bias: folded on host).
            for sb in range(XC // P):
                kb = xc * (XC // P) + sb
                ps = ps1.tile([P, FG], F32, tag="qkps")
                for db in range(DB):
                    nc.tensor.matmul(
                        ps,
                        xt[:, db, sb * P:(sb + 1) * P],
                        wv_sb[:, db, :],
                        start=(db == 0), stop=(db == DB - 1))
                nc.vector.tensor_copy(
                    cx.V[:, kb, :, 0:HD],
                    ps.rearrange("p (h c) -> p h c", h=HG))
                drip(per)

        # Tail: leftover dripped thunks (att(3)/proj(1)) interleaved with
        # attention of the last chunk; then proj(2..4) in emission order
        # (proj(ci) must be emitted after att(ci) - Tile orders deps by
        # emission). Junk warm matmuls hold 2.4 GHz.
        tail_att = _attention_chunk_thunks(nc, cx, NCH - 1, warm=True)
        while bg:
            if tail_att:
                tail_att.pop(0)()
            drip(1)
        tail_proj = _proj_chunk_thunks(nc, cx, NCH - 2, out_d, warm=True)
        k = max(1, len(tail_att) // max(1, len(tail_proj)))
        while tail_att or tail_proj:
            for _ in range(k):
                if tail_att:
                    tail_att.pop(0)()
            if tail_proj:
                tail_proj.pop(0)()
        for t in _proj_chunk_thunks(nc, cx, NCH - 1, out_d, warm=True):
            t()


def build_nc():
    nc = bacc.Bacc("TRN2", target_bir_lowering=False)
    xT_d = nc.dram_tensor("xT", [D, S], DT_MM, kind="ExternalInput")
    wq_d = nc.dram_tensor("wq", [D, FG], DT_MM, kind="ExternalInput")
    wk_d = nc.dram_tensor("wk", [D, FG], DT_MM, kind="ExternalInput")
    wv_d = nc.dram_tensor("wv", [D, FG], DT_MM, kind="ExternalInput")
    wp_d = nc.dram_tensor("wp", [FG, D], DT_MM, kind="ExternalInput")
    bq_d = nc.dram_tensor("bq", [FG], F32, kind="ExternalInput")
    out_d = nc.dram_tensor("out", [S, D], F32, kind="ExternalOutput")
    with tile.TileContext(nc) as tc:
        _body(tc, xT_d, wq_d, wk_d, wv_d, wp_d, bq_d, out_d)
    nc.compile()
    return nc


_NC = None


def _get_nc():
    global _NC
    if _NC is None:
        _NC = build_nc()
    return _NC


def make_in_maps(hs, w, bvec, pw):
    import ml_dtypes
    wdt = ml_dtypes.bfloat16 if DT_MM == BF16 else np.float32
    in_maps = []
    for core in range(NCORES):
        b, g = divmod(core, 2)
        lo, hi = g * FG, (g + 1) * FG
        in_maps.append({
            "xT": np.ascontiguousarray(hs[b].T).astype(wdt),
            "wq": np.ascontiguousarray(w[:, lo:hi]).astype(wdt),
            "wk": np.ascontiguousarray(w[:, D + lo:D + hi]).astype(wdt),
            "wv": np.ascontiguousarray(
                w[:, 2 * D + lo:2 * D + hi]).astype(wdt),
            "wp": np.ascontiguousarray(pw[lo:hi, :]).astype(wdt),
            "bq": np.ascontiguousarray(bvec[lo:hi]),
        })
    return in_maps


def combine(parts, bvec, pw, pb):
    bv = bvec[2 * D:3 * D].astype(np.float64)
    corr = (bv @ pw.astype(np.float64) + pb.astype(np.float64)).astype(
        np.float32)
    out = np.empty((B, S, D), np.float32)
    for b in range(B):
        out[b] = parts[2 * b] + parts[2 * b + 1] + corr
    return out


def kernel(hidden_states, c_attn_w, c_attn_b, c_proj_w, c_proj_b,
           **run_kwargs):
    hs = np.asarray(hidden_states, dtype=np.float32)
    w = np.asarray(c_attn_w, dtype=np.float32)
    bvec = np.asarray(c_attn_b, dtype=np.float32)
    pw = np.asarray(c_proj_w, dtype=np.float32)
    pb = np.asarray(c_proj_b, dtype=np.float32)
    nc = _get_nc()
    res = run_bass_kernel_spmd(nc, make_in_maps(hs, w, bvec, pw),
                               core_ids=list(range(NCORES)), **run_kwargs)
    parts = [res.results[i]["out"] for i in range(NCORES)]
    out = combine(parts, bvec, pw, pb)
    if run_kwargs:
        return out, res
    return out
